# revision 1
# baseline (speedup 1.0000x reference)
"""Trainium2 Bass kernel for a 6-layer GRU network (B=256, T=512, I=28, H=128, O=10).

Strategy: data-parallel across 8 NeuronCores (batch 256 -> 32 per core).
Per core, everything lives in "transposed" layout: partitions = hidden/gate
dim, free dim = time*batch. Per layer:
  - input projections gx = W_ih^T.T @ h_prev_seq computed as chunked GEMMs
    directly into PSUM (one bank per gate per chunk),
  - the sequential recurrence accumulates gh_r/gh_z onto gx_r/gx_z in PSUM
    via start=False matmuls; the n-gate recurrent part goes to a separate
    PSUM tile so r can gate it,
  - gates: sigmoid/tanh on ScalarE (per-layer biases ride the free ACT bias
    port), (gh_n + b_hh_n) * r fused into one scalar_tensor_tensor on DVE,
  - h_new = n + z*(h - n) on DVE, written into per-chunk h-seq SBUF tiles
    that the next layer's GEMM consumes.
Final: logits = h_last^T.T @ fc_w^T + fc_b (fc_b added via a K=1 accumulate
matmul), then log_softmax along the free dim.
"""

import numpy as np

H = 128
I_DIM = 28
L = 6
O = 10
B = 256
T = 512
NCORES = 8
PB = B // NCORES  # 32 batch rows per core
C = 16            # timesteps per chunk (C*PB = 512 = one PSUM bank of fp32)

_CACHE = {}


def _build(t_steps, dt_mm_name="float32"):
    from contextlib import ExitStack

    import concourse.bass as bass  # noqa: F401
    import concourse.tile as tile
    from concourse import bacc, mybir

    f32 = mybir.dt.float32
    dt_mm = getattr(mybir.dt, dt_mm_name)
    AF = mybir.ActivationFunctionType
    ALU = mybir.AluOpType

    n_chunks = t_steps // C
    assert n_chunks * C == t_steps

    nc = bacc.Bacc("TRN2", target_bir_lowering=False, debug=False)

    xT = nc.dram_tensor("xT", [I_DIM, PB * t_steps], dt_mm, kind="ExternalInput")
    wih0 = nc.dram_tensor("wih0", [I_DIM, 3 * H], dt_mm, kind="ExternalInput")
    wih = nc.dram_tensor("wih", [H, (L - 1) * 3 * H], dt_mm, kind="ExternalInput")
    whh = nc.dram_tensor("whh", [H, L * 3 * H], dt_mm, kind="ExternalInput")
    bias_r = nc.dram_tensor("bias_r", [H, L], f32, kind="ExternalInput")
    bias_z = nc.dram_tensor("bias_z", [H, L], f32, kind="ExternalInput")
    bihn = nc.dram_tensor("bihn", [H, L], f32, kind="ExternalInput")
    bhhn = nc.dram_tensor("bhhn", [H, L], f32, kind="ExternalInput")
    fcw = nc.dram_tensor("fcw", [H, O], dt_mm, kind="ExternalInput")
    fcb = nc.dram_tensor("fcb", [1, O], dt_mm, kind="ExternalInput")
    y = nc.dram_tensor("y", [PB, O], f32, kind="ExternalOutput")

    with tile.TileContext(nc) as tc, ExitStack() as ctx:
        consts = ctx.enter_context(tc.tile_pool(name="consts", bufs=1))
        hseq_pool = ctx.enter_context(tc.tile_pool(name="hseq", bufs=2 * n_chunks))
        gxr_pool = ctx.enter_context(tc.tile_pool(name="gxr", bufs=2, space="PSUM"))
        gxz_pool = ctx.enter_context(tc.tile_pool(name="gxz", bufs=2, space="PSUM"))
        gxn_pool = ctx.enter_context(tc.tile_pool(name="gxn", bufs=2, space="PSUM"))
        ps_small = ctx.enter_context(tc.tile_pool(name="ps_small", bufs=2, space="PSUM"))
        scratch = ctx.enter_context(tc.tile_pool(name="scratch", bufs=3))

        # --- load constants/weights ---
        xT_sb = consts.tile([I_DIM, PB * t_steps], dt_mm, tag="xT_sb")
        nc.gpsimd.dma_start(xT_sb[:], xT.ap())
        wih0_sb = consts.tile([I_DIM, 3 * H], dt_mm, tag="wih0_sb")
        nc.gpsimd.dma_start(wih0_sb[:], wih0.ap())
        wih_sb = consts.tile([H, (L - 1) * 3 * H], dt_mm, tag="wih_sb")
        nc.gpsimd.dma_start(wih_sb[:], wih.ap())
        whh_sb = consts.tile([H, L * 3 * H], dt_mm, tag="whh_sb")
        nc.gpsimd.dma_start(whh_sb[:], whh.ap())
        bias_r_sb = consts.tile([H, L], f32, tag="bias_r_sb")
        nc.gpsimd.dma_start(bias_r_sb[:], bias_r.ap())
        bias_z_sb = consts.tile([H, L], f32, tag="bias_z_sb")
        nc.gpsimd.dma_start(bias_z_sb[:], bias_z.ap())
        bihn_sb = consts.tile([H, L], f32, tag="bihn_sb")
        nc.gpsimd.dma_start(bihn_sb[:], bihn.ap())
        bhhn_sb = consts.tile([H, L], f32, tag="bhhn_sb")
        nc.gpsimd.dma_start(bhhn_sb[:], bhhn.ap())
        fcw_sb = consts.tile([H, O], dt_mm, tag="fcw_sb")
        nc.gpsimd.dma_start(fcw_sb[:], fcw.ap())
        fcb_sb = consts.tile([1, O], dt_mm, tag="fcb_sb")
        nc.gpsimd.dma_start(fcb_sb[:], fcb.ap())

        zeros_sb = consts.tile([H, PB], dt_mm, tag="zeros_sb")
        nc.vector.memset(zeros_sb[:], 0.0)
        ones_sb = consts.tile([1, PB], dt_mm, tag="ones_sb")
        nc.vector.memset(ones_sb[:], 1.0)

        def whh_g(layer, g):
            return whh_sb[:, (layer * 3 + g) * H:(layer * 3 + g + 1) * H]

        def wih_g(layer, g):
            assert layer >= 1
            base = ((layer - 1) * 3 + g) * H
            return wih_sb[:, base:base + H]

        prev_chunks = None  # list of SBUF tiles [H, C*PB] for layer l-1 output
        h_last = None
        for layer in range(L):
            cur_chunks = []
            h_prev = zeros_sb[:, :]
            for k in range(n_chunks):
                # --- input-projection GEMM for this chunk (into PSUM) ---
                gxr_t = gxr_pool.tile([H, C * PB], f32)
                gxz_t = gxz_pool.tile([H, C * PB], f32)
                gxn_t = gxn_pool.tile([H, C * PB], f32)
                if layer == 0:
                    mv = xT_sb[:, k * C * PB:(k + 1) * C * PB]
                    lhs = [wih0_sb[:, g * H:(g + 1) * H] for g in range(3)]
                else:
                    mv = prev_chunks[k][:, :]
                    lhs = [wih_g(layer, g) for g in range(3)]
                nc.tensor.matmul(gxr_t[:], lhs[0], mv, start=True, stop=False)
                nc.tensor.matmul(gxz_t[:], lhs[1], mv, start=True, stop=False)
                nc.tensor.matmul(gxn_t[:], lhs[2], mv, start=True, stop=True)

                hcur_t = hseq_pool.tile([H, C * PB], dt_mm)
                cur_chunks.append(hcur_t)

                for s in range(C):
                    sl = slice(s * PB, (s + 1) * PB)
                    # recurrent matmuls
                    nc.tensor.matmul(gxr_t[:, sl], whh_g(layer, 0), h_prev,
                                     start=False, stop=(s == C - 1),
                                     skip_group_check=True)
                    nc.tensor.matmul(gxz_t[:, sl], whh_g(layer, 1), h_prev,
                                     start=False, stop=(s == C - 1),
                                     skip_group_check=True)
                    ghn_t = ps_small.tile([H, PB], f32, tag="ghn")
                    nc.tensor.matmul(ghn_t[:], whh_g(layer, 2), h_prev,
                                     start=True, stop=True)
                    # gates
                    r_t = scratch.tile([H, PB], f32, tag="r")
                    nc.scalar.activation(r_t[:], gxr_t[:, sl], AF.Sigmoid,
                                         bias=bias_r_sb[:, layer:layer + 1])
                    z_t = scratch.tile([H, PB], f32, tag="z")
                    nc.scalar.activation(z_t[:], gxz_t[:, sl], AF.Sigmoid,
                                         bias=bias_z_sb[:, layer:layer + 1])
                    hn2_t = scratch.tile([H, PB], f32, tag="hn2")
                    nc.vector.scalar_tensor_tensor(
                        hn2_t[:], ghn_t[:], bhhn_sb[:, layer:layer + 1], r_t[:],
                        op0=ALU.add, op1=ALU.mult)
                    nin_t = scratch.tile([H, PB], f32, tag="nin")
                    nc.vector.tensor_tensor(nin_t[:], gxn_t[:, sl], hn2_t[:],
                                            op=ALU.add)
                    n_t = scratch.tile([H, PB], f32, tag="n")
                    nc.scalar.activation(n_t[:], nin_t[:], AF.Tanh,
                                         bias=bihn_sb[:, layer:layer + 1])
                    d_t = scratch.tile([H, PB], f32, tag="d")
                    nc.vector.tensor_tensor(d_t[:], h_prev, n_t[:],
                                            op=ALU.subtract)
                    e_t = scratch.tile([H, PB], f32, tag="e")
                    nc.vector.tensor_tensor(e_t[:], z_t[:], d_t[:], op=ALU.mult)
                    h_new = hcur_t[:, sl]
                    nc.vector.tensor_tensor(h_new, n_t[:], e_t[:], op=ALU.add)
                    h_prev = h_new
            prev_chunks = cur_chunks
            h_last = h_prev

        # --- FC + log_softmax on the last timestep of the last layer ---
        logits_ps = ps_small.tile([PB, O], f32, tag="ghn")
        nc.tensor.matmul(logits_ps[:], h_last, fcw_sb[:], start=True, stop=False)
        nc.tensor.matmul(logits_ps[:], ones_sb[:], fcb_sb[:],
                         start=False, stop=True, skip_group_check=True)
        mx_t = scratch.tile([PB, 1], f32, tag="mx")
        nc.vector.reduce_max(mx_t[:], logits_ps[:], axis=mybir.AxisListType.X)
        xm_t = scratch.tile([PB, O], f32, tag="xm")
        nc.vector.tensor_scalar(xm_t[:], logits_ps[:], mx_t[:], None,
                                op0=ALU.subtract)
        ex_t = scratch.tile([PB, O], f32, tag="ex")
        sum_t = scratch.tile([PB, 1], f32, tag="sum")
        nc.scalar.activation(ex_t[:], xm_t[:], AF.Exp, accum_out=sum_t[:])
        ls_t = scratch.tile([PB, 1], f32, tag="ls")
        nc.scalar.activation(ls_t[:], sum_t[:], AF.Ln)
        out_t = scratch.tile([PB, O], f32, tag="out")
        nc.vector.tensor_scalar(out_t[:], xm_t[:], ls_t[:], None,
                                op0=ALU.subtract)
        nc.gpsimd.dma_start(y.ap(), out_t[:])

    nc.compile()
    return nc


def _prep_inputs(x, W_ih0, W_ih_rest, W_hh, b_ih, b_hh, fc_w, fc_b, t_steps,
                 np_mm=np.float32):
    """Host-side reshape/transpose into the layouts the kernel expects."""
    f = np.float32
    b_ih = np.asarray(b_ih, f)
    b_hh = np.asarray(b_hh, f)
    shared = {
        "wih0": np.ascontiguousarray(np.asarray(W_ih0, f).T.astype(np_mm)),
        "wih": np.ascontiguousarray(
            np.concatenate([np.asarray(W_ih_rest[l], f).T for l in range(L - 1)],
                           axis=1).astype(np_mm)),
        "whh": np.ascontiguousarray(
            np.concatenate([np.asarray(W_hh[l], f).T for l in range(L)],
                           axis=1).astype(np_mm)),
        "bias_r": np.ascontiguousarray((b_ih[:, 0:H] + b_hh[:, 0:H]).T),
        "bias_z": np.ascontiguousarray((b_ih[:, H:2 * H] + b_hh[:, H:2 * H]).T),
        "bihn": np.ascontiguousarray(b_ih[:, 2 * H:3 * H].T),
        "bhhn": np.ascontiguousarray(b_hh[:, 2 * H:3 * H].T),
        "fcw": np.ascontiguousarray(np.asarray(fc_w, f).T.astype(np_mm)),
        "fcb": np.ascontiguousarray(np.asarray(fc_b, f).reshape(1, O).astype(np_mm)),
    }
    x = np.asarray(x, f)[:, :t_steps, :]
    in_maps = []
    for c in range(NCORES):
        xc = x[c * PB:(c + 1) * PB]                      # [PB, t, I]
        xT_c = np.ascontiguousarray(xc.transpose(2, 1, 0).reshape(I_DIM, t_steps * PB).astype(np_mm))
        in_maps.append({"xT": xT_c, **shared})
    return in_maps


def _run(nc, in_maps, trace=False):
    from concourse.bass_utils import run_bass_kernel_spmd
    return run_bass_kernel_spmd(nc, in_maps, core_ids=list(range(NCORES)),
                                trace=trace)


def kernel(x, W_ih0, W_ih_rest, W_hh, b_ih, b_hh, fc_w, fc_b):
    import ml_dtypes
    key = ("bf16", T)
    if key not in _CACHE:
        _CACHE[key] = _build(T, "bfloat16")
    nc = _CACHE[key]
    in_maps = _prep_inputs(x, W_ih0, W_ih_rest, W_hh, b_ih, b_hh, fc_w, fc_b, T,
                           np_mm=ml_dtypes.bfloat16)
    res = _run(nc, in_maps)
    return np.concatenate([res.results[c]["y"] for c in range(NCORES)], axis=0)

